# revision 24
# baseline (speedup 1.0000x reference)
"""Trainium2 Bass kernel for nn_Disentangler (gnn_message_passing).

Math (per timestamp t):
  nodes = LayerNorm(x[t, :40000, :])                      # [NT, 32]
  entire = scatter_add(nodes by indices into 50000 slots) # [NN, 32]
  h = gelu(entire_like.T @ mlp_w + mlp_b)                 # [2,16,1024]
  y = gelu(conv2d(h, 16x16 stride 16) + conv_b)           # [2,1,64]
  out[t] = y.transpose -> [1, 128]

Key reformulation: scatter_add followed by the dense matmul over node slots
equals a matmul over TOKENS with gathered weight rows:
    out[e, m] = sum_j nodes[j, e] * mlp_w[idx[j], m]
PSUM accumulation absorbs duplicate indices exactly, so no scatter is needed.
mlp_b is folded in via an extra weight row (50000) paired with a fake token
whose post-LN value is all-ones.

Weight rows are fetched with the ANT dma_gather instruction (512 rows +
1 sentinel per instruction, spread over 4 SWDGE queues). Its int16 index
limit is defeated by storing idx16 = n - 25000 and basing the source AP at
row 25000: the ucode's address math is signed (IVP_MULUSAN). The positive
sentinel as the final index defeats the trailing-negative truncation; pad
tokens alias the bias row (harmless: their lhsT columns are zero).

Sharding: data-parallel over timestamps - core k handles timestamps
{2k, 2k+1}; mlp weight replicated in bf16 (the TensorEngine consumes bf16
either way; storing bf16 halves HBM gather traffic).
"""
import numpy as np

# ---- problem constants (hardcoded per harness contract) ----
T, NTOK, E = 16, 65536, 32
NN, NT = 50000, 40000
C, K, M = 2, 16, 1024
NCORES = 8
T_LOC = T // NCORES          # 2 timestamps per core
P = 128
S = 316                      # token slots per partition, padded (40448 tokens)
NTP = S * P                  # 40960
BIAS_ROW = NN                # appended mlp_w row holding mlp_b
BIAS_TOK_P, BIAS_TOK_S = NT - (NT // P) * P, NT // P   # token 40000 -> (p=64, s=312)
IDX_BASE = 25000             # dma_gather source AP base row (signed int16 offsets)
B_CH = 4                     # weight chunks per dma_gather (512+1 indices)
NB = S // B_CH               # 80 gather batches per timestamp
NIDX = B_CH * P + 1          # 513 (sentinel keeps the last index positive)
SLOTS = (NIDX + 15) // 16    # 33 (int16 idx wrapped into 16 partitions)
EPS = 1e-5

_CACHE = {}
DEBUG = False


def _build(nc_mod):
    import concourse.bass as bass
    import concourse.bacc as bacc
    import concourse.tile as tile
    from concourse import mybir

    f32 = mybir.dt.float32
    bf16 = mybir.dt.bfloat16
    i16 = mybir.dt.int16

    nc = bacc.Bacc(target_bir_lowering=False, num_swdge_queues=4)
    x_in = nc.declare_dram_parameter("x", [T_LOC, NTP, E], bf16, isOutput=False)
    idx_in = nc.declare_dram_parameter("idx16", [T_LOC, P, NB, SLOTS], i16,
                                       isOutput=False)
    w_in = nc.declare_dram_parameter("w", [NN + 1, M], bf16, isOutput=False)
    lnw_in = nc.declare_dram_parameter("ln_w", [E], f32, isOutput=False)
    lnb_in = nc.declare_dram_parameter("ln_b", [E], f32, isOutput=False)
    cw_in = nc.declare_dram_parameter("conv_w", [C, C, K, K], f32, isOutput=False)
    cb_in = nc.declare_dram_parameter("conv_b", [C], f32, isOutput=False)
    out_d = nc.declare_dram_parameter("out", [T_LOC, C * (M // K)], f32, isOutput=True)

    def bcast_inner(ap2, n):
        # [P, S] -> [P, S, n] with 0-stride inner dim
        return bass.AP(tensor=ap2.tensor, offset=ap2.offset,
                       ap=[ap2.ap[0], ap2.ap[1], [0, n]])

    def bcast_mid(ap2, n):
        # [P, E] -> [P, n, E] with 0-stride middle dim
        return bass.AP(tensor=ap2.tensor, offset=ap2.offset,
                       ap=[ap2.ap[0], [0, n], ap2.ap[1]])

    with tile.TileContext(nc) as tc:
        import contextlib
        ctx = contextlib.ExitStack()
        with ctx:
            consts = ctx.enter_context(tc.tile_pool(name="consts", bufs=1))
            xpool = ctx.enter_context(tc.tile_pool(name="xp", bufs=4))
            xcpool = ctx.enter_context(tc.tile_pool(name="xcp", bufs=2))
            ndpool = ctx.enter_context(tc.tile_pool(name="ndp", bufs=8))
            stpool = ctx.enter_context(tc.tile_pool(name="stp", bufs=4))
            ipool = ctx.enter_context(tc.tile_pool(name="ip", bufs=2))
            wpool = ctx.enter_context(tc.tile_pool(name="wp", bufs=8))
            epool = ctx.enter_context(tc.tile_pool(name="ep", bufs=2))
            pspool = ctx.enter_context(tc.tile_pool(name="ps", bufs=2, space="PSUM"))
            ps2pool = ctx.enter_context(tc.tile_pool(name="ps2", bufs=2, space="PSUM"))

            # --- constants ---
            lnw_ap, lnb_ap = lnw_in[:], lnb_in[:]
            cw_ap, cb_ap = cw_in[:], cb_in[:]
            lnw_sb = consts.tile([P, E], f32)
            nc.gpsimd.dma_start(out=lnw_sb[:], in_=bass.AP(
                tensor=lnw_ap.tensor, offset=lnw_ap.offset,
                ap=[[0, P], [1, E]]))
            lnb_sb = consts.tile([P, E], f32)
            nc.gpsimd.dma_start(out=lnb_sb[:], in_=bass.AP(
                tensor=lnb_ap.tensor, offset=lnb_ap.offset,
                ap=[[0, P], [1, E]]))
            # conv weights: partition (i,kh) (stride 16), free [kw, o]
            cw_sb = consts.tile([C * K, K, C], bf16)
            nc.gpsimd.dma_start(out=cw_sb[:], in_=bass.AP(
                tensor=cw_ap.tensor, offset=cw_ap.offset,
                ap=[[K, C * K], [1, K], [C * K * K, C]]))
            cb_sb = consts.tile([C, 1], f32)
            nc.gpsimd.dma_start(out=cb_sb[:], in_=bass.AP(
                tensor=cb_ap.tensor, offset=cb_ap.offset,
                ap=[[1, C], [0, 1]]))

            QS = S // 4          # layernorm processed in quarters so matmuls
            for t in range(T_LOC):   # can start before the whole LN finishes
                # --- load: idx first (gathers depend on it; HWDGE is FIFO) ---
                idx_t = ipool.tile([P, NB, SLOTS], i16)
                nc.sync.dma_start(out=idx_t[:], in_=idx_in[:][t])
                x_re = x_in[:].rearrange("t (s p) e -> t p s e", p=P)[t]

                # --- layernorm over E, quarter by quarter (x loaded per
                # quarter as separate tiles so the first matmuls start early) ---
                nd_q = []
                for q in range(4):
                    sl = slice(q * QS, (q + 1) * QS)
                    xq_t = xpool.tile([P, QS, E], bf16, tag="xq", name=f"x_{t}_{q}")
                    nc.sync.dma_start(out=xq_t[:], in_=x_re[:, sl, :])
                    # convert to f32 for the LN arithmetic (DVE is not critical)
                    xf = xcpool.tile([P, QS, E], f32, tag="xf", name=f"xf_{t}_{q}")
                    nc.vector.tensor_copy(xf[:], xq_t[:])
                    xq = xf[:]
                    sum_t = stpool.tile([P, QS], f32)
                    nc.vector.tensor_reduce(out=sum_t[:], in_=xq,
                                            axis=mybir.AxisListType.X,
                                            op=mybir.AluOpType.add)
                    nc.vector.tensor_scalar_mul(sum_t[:], sum_t[:], -1.0 / E)
                    xc = xcpool.tile([P, QS, E], f32)
                    nc.vector.tensor_tensor(out=xc[:], in0=xq,
                                            in1=bcast_inner(sum_t[:], E),
                                            op=mybir.AluOpType.add)
                    # xf is dead now; reuse it as square scratch
                    nc.vector.tensor_tensor(out=xq, in0=xc[:], in1=xc[:],
                                            op=mybir.AluOpType.mult)
                    var_t = stpool.tile([P, QS], f32)
                    nc.vector.tensor_reduce(out=var_t[:], in_=xq,
                                            axis=mybir.AxisListType.X,
                                            op=mybir.AluOpType.add)
                    nc.vector.tensor_scalar(out=var_t[:], in0=var_t[:],
                                            scalar1=1.0 / E, scalar2=EPS,
                                            op0=mybir.AluOpType.mult,
                                            op1=mybir.AluOpType.add)
                    std_t = stpool.tile([P, QS], f32)
                    nc.scalar.activation(out=std_t[:], in_=var_t[:],
                                         func=mybir.ActivationFunctionType.Sqrt)
                    rstd_t = stpool.tile([P, QS], f32)
                    nc.vector.reciprocal(out=rstd_t[:], in_=std_t[:])
                    nc.vector.tensor_tensor(out=xc[:], in0=xc[:],
                                            in1=bcast_inner(rstd_t[:], E),
                                            op=mybir.AluOpType.mult)
                    nc.vector.tensor_tensor(out=xc[:], in0=xc[:],
                                            in1=bcast_mid(lnw_sb[:], QS),
                                            op=mybir.AluOpType.mult)
                    nodes = ndpool.tile([P, QS, E], bf16, tag="ndq",
                                        name=f"nodes_{t}_{q}")
                    nc.vector.tensor_tensor(out=nodes[:], in0=xc[:],
                                            in1=bcast_mid(lnb_sb[:], QS),
                                            op=mybir.AluOpType.add)
                    nd_q.append(nodes)
                # zero the pad-token slots (LN maps zero rows to ln_b), then
                # set the fake all-ones token pairing with the mlp_b weight row
                nd3 = nd_q[3]
                bs = BIAS_TOK_S - 3 * QS
                nc.vector.memset(nd3[BIAS_TOK_P:P, bs:bs + 1, :], 0.0)
                nc.vector.memset(nd3[:, bs + 1:QS, :], 0.0)
                nc.vector.memset(
                    nd3[BIAS_TOK_P:BIAS_TOK_P + 1, bs:bs + 1, :], 1.0)

                # --- token-contraction matmul with batch-gathered weight rows ---
                ps_h = [pspool.tile([E, 512], f32, tag=f"ps{h}", name=f"ps_{t}_{h}")
                        for h in range(M // 512)]
                for b in range(NB):
                    wt = wpool.tile([P, B_CH + 1, M], bf16)
                    nc.gpsimd.dma_gather(
                        out_ap=wt[:],
                        in_ap=w_in[IDX_BASE:, :],
                        idxs_ap=idx_t[:, b, :],
                        num_idxs=NIDX,
                        num_idxs_reg=NIDX,
                        elem_size=M,
                        queue_num=(t * NB + b) % 4,
                    )
                    for c4 in range(B_CH):
                        cg = b * B_CH + c4
                        for h in range(M // 512):
                            nc.tensor.matmul(out=ps_h[h][:],
                                             lhsT=nd_q[cg // QS][:, cg % QS, :],
                                             rhs=wt[:, c4, h * 512:(h + 1) * 512],
                                             start=(cg == 0), stop=(cg == S - 1))

                # --- epilogue: gelu -> conv(16x16/16) -> +bias -> gelu ---
                gelu_sb = epool.tile([E, M], bf16)
                for h in range(M // 512):
                    nc.scalar.activation(out=gelu_sb[:, h * 512:(h + 1) * 512],
                                         in_=ps_h[h][:],
                                         func=mybir.ActivationFunctionType.Gelu)
                ps2 = ps2pool.tile([C, M // K], f32)
                g_r = gelu_sb[:].rearrange("p (w k) -> p k w", k=K)
                for kw in range(K):
                    nc.tensor.matmul(out=ps2[:], lhsT=cw_sb[:, kw, :],
                                     rhs=g_r[:, kw, :],
                                     start=(kw == 0), stop=(kw == K - 1))
                y_sb = epool.tile([C, M // K], f32)
                nc.vector.tensor_tensor(out=y_sb[:], in0=ps2[:],
                                        in1=bcast_inner(cb_sb[:], M // K),
                                        op=mybir.AluOpType.add)
                y2_sb = epool.tile([C, M // K], f32)
                nc.scalar.activation(out=y2_sb[:], in_=y_sb[:],
                                     func=mybir.ActivationFunctionType.Gelu)
                nc.sync.dma_start(
                    out=out_d[:].rearrange("t (o w) -> t o w", o=C)[t], in_=y2_sb[:])

    nc.compile()
    return nc


def kernel(x, ln_w, ln_b, mlp_w, mlp_b, conv_w, conv_b, indices_subnodes,
           n_node_tokens):
    from concourse.bass_utils import run_bass_kernel_spmd
    import ml_dtypes

    x = np.asarray(x)
    idx = np.asarray(indices_subnodes)
    nt = int(n_node_tokens)
    assert nt == NT, nt

    if "nc" not in _CACHE:
        _CACHE["nc"] = _build(None)
    nc = _CACHE["nc"]

    # weight augmented with the bias row; stored bf16 (the kernel consumes the
    # weight in bf16 on the TensorEngine either way - this halves HBM traffic)
    w_aug = np.concatenate([np.asarray(mlp_w, np.float32),
                            np.asarray(mlp_b, np.float32)[None, :]],
                           axis=0).astype(ml_dtypes.bfloat16)

    # pad tokens per timestamp: zero values; their int16 offsets point at the
    # (positive) bias row so the trailing-negative truncation never triggers
    x_pad = np.zeros((T, NTP, E), ml_dtypes.bfloat16)
    x_pad[:, :NT, :] = x[:, :NT, :].astype(ml_dtypes.bfloat16)

    # signed int16 gather offsets, wrapped [list pos i -> partition i%16,
    # slot i//16] and replicated to all 8 Q7 core groups
    flat = np.full((T, NTP), BIAS_ROW - IDX_BASE, np.int16)
    flat[:, :NT] = (idx.astype(np.int32) - IDX_BASE).astype(np.int16)
    off = np.full((T, NB, SLOTS * 16), BIAS_ROW - IDX_BASE, np.int16)
    off[:, :, :B_CH * P] = flat.reshape(T, NB, B_CH * P)
    wrapped = off.reshape(T, NB, SLOTS, 16).transpose(0, 3, 1, 2)  # [T,16,NB,SLOTS]
    idx16 = np.tile(wrapped, (1, 8, 1, 1))                          # [T,128,NB,SLOTS]

    in_maps = []
    for k in range(NCORES):
        sl = slice(k * T_LOC, (k + 1) * T_LOC)
        in_maps.append({
            "x": x_pad[sl],
            "idx16": idx16[sl],
            "w": w_aug,
            "ln_w": np.asarray(ln_w, np.float32),
            "ln_b": np.asarray(ln_b, np.float32),
            "conv_w": np.asarray(conv_w, np.float32),
            "conv_b": np.asarray(conv_b, np.float32),
        })
    res = run_bass_kernel_spmd(nc, in_maps, core_ids=list(range(NCORES)))
    out = np.concatenate([res.results[k]["out"] for k in range(NCORES)], axis=0)
    return out.reshape(T, 1, C * (M // K))
